# revision 15
# baseline (speedup 1.0000x reference)
"""Trilinear interpolation (DeformationGrid) Bass kernel for 8 trn2 NeuronCores.

Data-parallel: coords/output sharded along the point axis across 8 cores;
theta replicated. Per core:
  1. Build a 2x2x2-corner block table in HBM (fp16): row b=(i*127+j)*127+k
     holds theta[i..i+1, j..j+1, k..k+1, :] (24 halves = 48B):
       - A16 slab: SWDGE cast-load (fp32 HBM -> fp16 SBUF), partition j =
         y-row j of x-slices [i0..i0+inn].
       - S16 slab: SWDGE SBUF->SBUF partition-shift (S16[j] = A16[j+1]) so
         y+1 data sits in partition j (DVE lanes can't cross partitions).
       - 4 strided DVE copies interleave (x,y)-corner segments into OT rows;
         a SWDGE DMA writes the rows to the HBM table.
     All bulk DMAs go through SWDGE (gpsimd): on this runtime the HWDGE
     path funnels every data descriptor through a single SDMA engine
     (~30 GB/s) while the other 15 engines' sem increments fire with no
     data behind them, releasing waiters early. SWDGE spreads descriptors
     across all 16 engines and each engine's increment follows its data.
  2. Stream points: cell ids + trilinear corner weights (DVE; fr=mod(xf,1)
     floor trick), ACT expands the 8 corner weights to 24 (one per gathered
     element), per-point 48B indirect-DMA gathers (one idx per partition
     per call - the HW consumes only the first index per partition), DVE
     fully-packed fp16 multiply + pairwise add tree.
Per-slot-parity DMA semaphores + issue gating make completion waits
unambiguous (a wait value can only be crossed by the DMA it guards).
Raw-bass implementation (manual semaphores).
"""
import sys

sys.path.insert(0, "/opt/trn_rl_repo")

import numpy as np

from concourse import bass, mybir
from concourse.bass import AP
from concourse.bass_utils import run_bass_kernel_spmd

N_CORES = 8
N_TOTAL = 4194304
NP = N_TOTAL // N_CORES        # 524288 points per core
G = 128
GD = G - 1                     # 127
NBLK = GD * GD * GD            # 2048383 block rows
ENT = 24                       # fp16 elems per block row (48B)
P = 128
B = 256                        # points per partition per stream tile
IW = 4                         # x-slices per build tile
WARM = 64                      # SWDGE warm-up gathers (cold-ring guard)

F16 = mybir.dt.float16
F32 = mybir.dt.float32
I32 = mybir.dt.int32
OP = mybir.AluOpType
ACTF = mybir.ActivationFunctionType


def mkap(t_ap: AP, offset_elems: int, dims) -> AP:
    return AP(tensor=t_ap.tensor, offset=t_ap.offset + offset_elems, ap=list(dims))


def build_program(np_points: int = NP) -> bass.Bass:
    nc = bass.Bass()
    coords_in = nc.declare_dram_parameter("coords", [np_points, 3], F32, isOutput=False)
    theta_in = nc.declare_dram_parameter("theta", [G * G * G, 3], F32, isOutput=False)
    out_ext = nc.declare_dram_parameter("out", [np_points, 3], F32, isOutput=True)
    tbl = nc.dram_tensor("tbl", [NBLK, ENT], F16)

    T = np_points // (P * B)
    assert T * P * B == np_points
    bt = [(i0, min(IW, GD - i0)) for i0 in range(0, GD, IW)]
    NTB = len(bt)
    LDW = (IW + 1) * G * 3     # elems per partition per loaded slab

    from contextlib import ExitStack
    with ExitStack() as _ctx:
        A16 = _ctx.enter_context(nc.sbuf_tensor([P, 2 * LDW], F16))
        S16 = _ctx.enter_context(nc.sbuf_tensor([P, 2 * LDW], F16))
        OT = _ctx.enter_context(nc.sbuf_tensor([P, 2 * IW * GD * ENT], F16))
        CO = _ctx.enter_context(nc.sbuf_tensor([P, 2 * B * 3], F32))
        XF = _ctx.enter_context(nc.sbuf_tensor([P, 2 * B * 3], F32))
        FR = _ctx.enter_context(nc.sbuf_tensor([P, 2 * B * 3], F32))
        I0 = _ctx.enter_context(nc.sbuf_tensor([P, 2 * B * 3], F32))
        T1 = _ctx.enter_context(nc.sbuf_tensor([P, 2 * B], F32))
        T2 = _ctx.enter_context(nc.sbuf_tensor([P, 2 * B], F32))
        IDX = _ctx.enter_context(nc.sbuf_tensor([P, 2 * B], I32))
        W2X = _ctx.enter_context(nc.sbuf_tensor([P, 2 * B * 2], F16))
        W2Y = _ctx.enter_context(nc.sbuf_tensor([P, 2 * B * 2], F16))
        W2Z = _ctx.enter_context(nc.sbuf_tensor([P, 2 * B * 2], F16))
        M4 = _ctx.enter_context(nc.sbuf_tensor([P, 2 * B * 4], F16))
        W8 = _ctx.enter_context(nc.sbuf_tensor([P, 2 * B * 8], F16))
        W24 = _ctx.enter_context(nc.sbuf_tensor([P, 2 * B * ENT], F16))
        GT = _ctx.enter_context(nc.sbuf_tensor([P, 2 * B * ENT], F16))
        PR = _ctx.enter_context(nc.sbuf_tensor([P, B * ENT], F16))
        Q2 = _ctx.enter_context(nc.sbuf_tensor([P, B * 12], F16))
        R2 = _ctx.enter_context(nc.sbuf_tensor([P, B * 6], F16))
        O3 = _ctx.enter_context(nc.sbuf_tensor([P, 2 * B * 3], F32))
        TG = _ctx.enter_context(nc.sbuf_tensor([P, 2 * B * 3], F32))
        dmaL = [_ctx.enter_context(nc.semaphore(f"dmaL{i}")) for i in range(2)]
        dmaS = [_ctx.enter_context(nc.semaphore(f"dmaS{i}")) for i in range(2)]
        dmaW = _ctx.enter_context(nc.semaphore("dmaW"))
        dmaC = [_ctx.enter_context(nc.semaphore(f"dmaC{i}")) for i in range(2)]
        dmaO = [_ctx.enter_context(nc.semaphore(f"dmaO{i}")) for i in range(2)]
        dmaG = [_ctx.enter_context(nc.semaphore(f"dmaG{i}")) for i in range(8)]
        vec_b = _ctx.enter_context(nc.semaphore("vec_b"))
        v_fr = _ctx.enter_context(nc.semaphore("v_fr"))
        v_idx = _ctx.enter_context(nc.semaphore("v_idx"))
        v_w8 = _ctx.enter_context(nc.semaphore("v_w8"))
        act_s = _ctx.enter_context(nc.semaphore("act_s"))
        v_xf = _ctx.enter_context(nc.semaphore("v_xf"))
        v_o3 = _ctx.enter_context(nc.semaphore("v_o3"))
        dmaWm = _ctx.enter_context(nc.semaphore("dmaWm"))
        block = _ctx.enter_context(nc.Block())

        @block.sync
        def _(sync):
            # stream coords loads + output writes (HWDGE is slow on this
            # runtime but these are small and fully hidden under the
            # per-tile gather window)
            for t in range(T):
                if t >= 2:
                    sync.wait_ge(v_xf, t - 1)
                sync.dma_start(
                    out=mkap(CO[:], (t % 2) * B * 3, [[2 * B * 3, P], [1, B * 3]]),
                    in_=mkap(coords_in[:], t * P * B * 3, [[B * 3, P], [1, B * 3]]),
                ).then_inc(dmaC[t % 2], 16)
                if t >= 1:
                    sync.wait_ge(v_o3, t)
                    sync.dma_start(
                        out=mkap(out_ext[:], (t - 1) * P * B * 3,
                                 [[B * 3, P], [1, B * 3]]),
                        in_=mkap(O3[:], ((t - 1) % 2) * B * 3,
                                 [[2 * B * 3, P], [1, B * 3]]),
                    ).then_inc(dmaO[(t - 1) % 2], 16)
            sync.wait_ge(v_o3, T)
            sync.dma_start(
                out=mkap(out_ext[:], (T - 1) * P * B * 3, [[B * 3, P], [1, B * 3]]),
                in_=mkap(O3[:], ((T - 1) % 2) * B * 3, [[2 * B * 3, P], [1, B * 3]]),
            ).then_inc(dmaO[(T - 1) % 2], 16)
            sync.wait_ge(dmaO[(T - 1) % 2], 16 * ((T + 1) // 2))

        @block.scalar
        def _(scalar):
            # stream: W2[:, 0::2] = 1 - frac, W2[:, 1::2] = frac, then the
            # 8 corner weights expanded x3 into per-element weights W24
            for t in range(T):
                s3 = (t % 2) * B * 3
                s2 = (t % 2) * B * 2
                s8 = (t % 2) * B * 8
                sE = (t % 2) * B * ENT
                scalar.wait_ge(v_fr, t + 1)
                if t >= 2:
                    scalar.wait_ge(v_w8, t - 1)
                for c, W2 in ((0, W2X), (1, W2Y), (2, W2Z)):
                    nc.scalar.activation(
                        mkap(W2[:], s2, [[2 * B * 2, P], [2, B]]),
                        mkap(FR[:], s3 + c, [[2 * B * 3, P], [3, B]]),
                        ACTF.Copy, bias=1.0, scale=-1.0,
                    ).then_inc(act_s, 1)
                    nc.scalar.activation(
                        mkap(W2[:], s2 + 1, [[2 * B * 2, P], [2, B]]),
                        mkap(FR[:], s3 + c, [[2 * B * 3, P], [3, B]]),
                        ACTF.Copy,
                    ).then_inc(act_s, 1)
                scalar.wait_ge(v_w8, t + 1)
                if t >= 2:
                    scalar.wait_ge(v_o3, t - 1)
                nc.scalar.activation(
                    mkap(W24[:], sE, [[2 * B * ENT, P], [1, B * ENT]]),
                    mkap(W8[:], s8, [[2 * B * 8, P], [1, B * 8], [0, 3]]),
                    ACTF.Copy,
                ).then_inc(act_s, 1)

        @block.vector
        def _(vector):
            for bi, (i0, inn) in enumerate(bt):
                s = (bi % 2) * LDW
                so = (bi % 2) * IW * GD * ENT
                vector.wait_ge(dmaS[bi % 2], 16 * (bi // 2 + 1))
                if bi >= 2:
                    vector.wait_ge(dmaW, 16 * (bi - 1))
                for di, dj in ((0, 0), (0, 1), (1, 0), (1, 1)):
                    srcbuf = (A16 if dj == 0 else S16)
                    nc.vector.tensor_copy(
                        mkap(OT[:], so + (di * 2 + dj) * 6,
                             [[2 * IW * GD * ENT, GD], [GD * ENT, inn], [ENT, GD], [1, 6]]),
                        mkap(srcbuf[:], s + di * (G * 3),
                             [[2 * LDW, GD], [G * 3, inn], [3, GD], [1, 6]]),
                    ).then_inc(vec_b, 1)
            for t in range(T):
                s3 = (t % 2) * B * 3
                s1 = (t % 2) * B
                s2 = (t % 2) * B * 2
                s4 = (t % 2) * B * 4
                s8 = (t % 2) * B * 8
                vector.wait_ge(dmaC[t % 2], 16 * (t // 2 + 1))
                if t >= 2:
                    # IDX/GT slot reuse: gathers of t-2 must be fully issued
                    vector.wait_ge(dmaG[(t - 2) % 8], 16 * B * ((t - 2) // 8 + 1))
                if t >= 3:
                    # O3 slot reuse: the out-write that last read this slot
                    # (issued at sync iteration t-2) must be done
                    vector.wait_ge(dmaO[(t - 1) % 2],
                                   16 * ((t - 3 - ((t - 1) % 2)) // 2 + 1))
                co = mkap(CO[:], s3, [[2 * B * 3, P], [1, B * 3]])
                xf = mkap(XF[:], s3, [[2 * B * 3, P], [1, B * 3]])
                fr = mkap(FR[:], s3, [[2 * B * 3, P], [1, B * 3]])
                i0v = mkap(I0[:], s3, [[2 * B * 3, P], [1, B * 3]])
                tg = mkap(TG[:], s3, [[2 * B * 3, P], [1, B * 3]])
                nc.vector.tensor_scalar(xf, co, float(GD), None, OP.mult).then_inc(v_xf, 1)
                # floor(xf): round via magic number, then subtract (rounded > xf)
                nc.vector.tensor_scalar(i0v, xf, 8388608.0, 8388608.0, OP.add, OP.subtract)
                nc.vector.tensor_tensor(tg, i0v, xf, OP.is_gt)
                nc.vector.tensor_tensor(i0v, i0v, tg, OP.subtract)
                nc.vector.tensor_tensor(fr, xf, i0v, OP.subtract).then_inc(v_fr, 1)
                i03 = [mkap(I0[:], s3 + c, [[2 * B * 3, P], [3, B]]) for c in range(3)]
                t1 = mkap(T1[:], s1, [[2 * B, P], [1, B]])
                t2 = mkap(T2[:], s1, [[2 * B, P], [1, B]])
                idx = mkap(IDX[:], s1, [[2 * B, P], [1, B]])
                nc.vector.scalar_tensor_tensor(t1, i03[0], float(GD), i03[1], OP.mult, OP.add)
                nc.vector.scalar_tensor_tensor(t2, t1, float(GD), i03[2], OP.mult, OP.add)
                nc.vector.tensor_scalar(t2, t2, 0.0, float(NBLK - 1), OP.max, OP.min)
                nc.vector.tensor_copy(idx, t2).then_inc(v_idx, 1)
                # corner weight products: M4 = wx (x) wy, W8 = M4 (x) wz
                vector.wait_ge(act_s, 7 * t + 6)
                nc.vector.tensor_tensor(
                    mkap(M4[:], s4, [[2 * B * 4, P], [1, B * 4]]),
                    mkap(W2X[:], s2, [[2 * B * 2, P], [2, B], [1, 2], [0, 2]]),
                    mkap(W2Y[:], s2, [[2 * B * 2, P], [2, B], [0, 2], [1, 2]]),
                    OP.mult)
                nc.vector.tensor_tensor(
                    mkap(W8[:], s8, [[2 * B * 8, P], [1, B * 8]]),
                    mkap(M4[:], s4, [[2 * B * 4, P], [4, B], [1, 4], [0, 2]]),
                    mkap(W2Z[:], s2, [[2 * B * 2, P], [2, B], [0, 4], [1, 2]]),
                    OP.mult).then_inc(v_w8, 1)
                if t >= 1:
                    tp = t - 1
                    sEp = (tp % 2) * B * ENT
                    s3p = (tp % 2) * B * 3
                    vector.wait_ge(dmaG[tp % 8], 16 * B * (tp // 8 + 1))
                    vector.wait_ge(act_s, 7 * (tp + 1))
                    nc.vector.tensor_tensor(
                        mkap(PR[:], 0, [[B * ENT, P], [1, B * ENT]]),
                        mkap(GT[:], sEp, [[2 * B * ENT, P], [1, B * ENT]]),
                        mkap(W24[:], sEp, [[2 * B * ENT, P], [1, B * ENT]]),
                        OP.mult)
                    # pairwise add tree over the 8 corners (xy pairs, then z)
                    nc.vector.tensor_tensor(
                        mkap(Q2[:], 0, [[B * 12, P], [1, B * 12]]),
                        mkap(PR[:], 0, [[B * ENT, P], [ENT, B], [1, 12]]),
                        mkap(PR[:], 12, [[B * ENT, P], [ENT, B], [1, 12]]),
                        OP.add)
                    nc.vector.tensor_tensor(
                        mkap(R2[:], 0, [[B * 6, P], [1, B * 6]]),
                        mkap(Q2[:], 0, [[B * 12, P], [12, B], [1, 6]]),
                        mkap(Q2[:], 6, [[B * 12, P], [12, B], [1, 6]]),
                        OP.add)
                    nc.vector.tensor_tensor(
                        mkap(O3[:], s3p, [[2 * B * 3, P], [1, B * 3]]),
                        mkap(R2[:], 0, [[B * 6, P], [6, B], [1, 3]]),
                        mkap(R2[:], 3, [[B * 6, P], [6, B], [1, 3]]),
                        OP.add).then_inc(v_o3, 1)
            tp = T - 1
            sEp = (tp % 2) * B * ENT
            s3p = (tp % 2) * B * 3
            vector.wait_ge(dmaG[tp % 8], 16 * B * (tp // 8 + 1))
            vector.wait_ge(act_s, 7 * (tp + 1))
            nc.vector.tensor_tensor(
                mkap(PR[:], 0, [[B * ENT, P], [1, B * ENT]]),
                mkap(GT[:], sEp, [[2 * B * ENT, P], [1, B * ENT]]),
                mkap(W24[:], sEp, [[2 * B * ENT, P], [1, B * ENT]]),
                OP.mult)
            nc.vector.tensor_tensor(
                mkap(Q2[:], 0, [[B * 12, P], [1, B * 12]]),
                mkap(PR[:], 0, [[B * ENT, P], [ENT, B], [1, 12]]),
                mkap(PR[:], 12, [[B * ENT, P], [ENT, B], [1, 12]]),
                OP.add)
            nc.vector.tensor_tensor(
                mkap(R2[:], 0, [[B * 6, P], [1, B * 6]]),
                mkap(Q2[:], 0, [[B * 12, P], [12, B], [1, 6]]),
                mkap(Q2[:], 6, [[B * 12, P], [12, B], [1, 6]]),
                OP.add)
            nc.vector.tensor_tensor(
                mkap(O3[:], s3p, [[2 * B * 3, P], [1, B * 3]]),
                mkap(R2[:], 0, [[B * 6, P], [6, B], [1, 3]]),
                mkap(R2[:], 3, [[B * 6, P], [6, B], [1, 3]]),
                OP.add).then_inc(v_o3, 1)

        @block.gpsimd
        def _(gpsimd):
            # build: SWDGE cast-load A16, SBUF-shift S16, table write
            for bi, (i0, inn) in enumerate(bt):
                nld = inn + 1
                s = (bi % 2) * LDW
                if bi >= 2:
                    gpsimd.wait_ge(vec_b, 4 * (bi - 1))
                gpsimd.dma_start(
                    out=mkap(A16[:], s, [[2 * LDW, P], [G * 3, nld], [1, G * 3]]),
                    in_=mkap(theta_in[:], i0 * G * G * 3,
                             [[G * 3, P], [G * G * 3, nld], [1, G * 3]]),
                ).then_inc(dmaL[bi % 2], 16)
                gpsimd.wait_ge(dmaL[bi % 2], 16 * (bi // 2 + 1))
                gpsimd.dma_start(
                    out=mkap(S16[:], s, [[2 * LDW, GD], [1, nld * G * 3]]),
                    in_=mkap(A16[:], s + 2 * LDW, [[2 * LDW, GD], [1, nld * G * 3]]),
                ).then_inc(dmaS[bi % 2], 16)
                if bi >= 1:
                    p0, pn = bt[bi - 1]
                    gpsimd.wait_ge(vec_b, 4 * bi)
                    so = ((bi - 1) % 2) * IW * GD * ENT
                    gpsimd.dma_start(
                        out=mkap(tbl[:], p0 * GD * GD * ENT,
                                 [[GD * ENT, GD], [GD * GD * ENT, pn], [1, GD * ENT]]),
                        in_=mkap(OT[:], so,
                                 [[2 * IW * GD * ENT, GD], [GD * ENT, pn], [1, GD * ENT]]),
                    ).then_inc(dmaW, 16)
            p0, pn = bt[NTB - 1]
            gpsimd.wait_ge(vec_b, 4 * NTB)
            so = ((NTB - 1) % 2) * IW * GD * ENT
            gpsimd.dma_start(
                out=mkap(tbl[:], p0 * GD * GD * ENT,
                         [[GD * ENT, GD], [GD * GD * ENT, pn], [1, GD * ENT]]),
                in_=mkap(OT[:], so,
                         [[2 * IW * GD * ENT, GD], [GD * ENT, pn], [1, GD * ENT]]),
            ).then_inc(dmaW, 16)
            gpsimd.wait_ge(dmaW, 16 * NTB)
            # SWDGE warm-up: duplicate tile-0 gathers (cold rings corrupt
            # the first batch on engine 0 otherwise); real tile 0 follows
            gpsimd.wait_ge(v_idx, 1)
            for k in range(WARM):
                gpsimd.indirect_dma_start(
                    out=mkap(GT[:], k * ENT, [[2 * B * ENT, P], [1, ENT]]),
                    out_offset=None,
                    in_=tbl[:],
                    in_offset=bass.IndirectOffsetOnAxis(
                        ap=mkap(IDX[:], k, [[2 * B, P], [1, 1]]), axis=0),
                ).then_inc(dmaWm, 16)
            gpsimd.wait_ge(dmaWm, 16 * WARM)
            for t in range(T):
                sE = (t % 2) * B * ENT
                s1 = (t % 2) * B
                gpsimd.wait_ge(v_idx, t + 1)
                if t >= 2:
                    gpsimd.wait_ge(v_o3, t - 1)
                for k in range(B):
                    gpsimd.indirect_dma_start(
                        out=mkap(GT[:], sE + k * ENT, [[2 * B * ENT, P], [1, ENT]]),
                        out_offset=None,
                        in_=tbl[:],
                        in_offset=bass.IndirectOffsetOnAxis(
                            ap=mkap(IDX[:], s1 + k, [[2 * B, P], [1, 1]]), axis=0),
                    ).then_inc(dmaG[t % 8], 16)
    return nc


_CACHED = {}


def _get_program():
    if "nc" not in _CACHED:
        _CACHED["nc"] = build_program()
    return _CACHED["nc"]


def kernel(coords: np.ndarray, theta: np.ndarray) -> np.ndarray:
    coords = np.ascontiguousarray(coords, dtype=np.float32)
    theta = np.ascontiguousarray(theta, dtype=np.float32).reshape(G * G * G, 3)
    nc = _get_program()
    shards = coords.reshape(N_CORES, NP, 3)
    in_maps = [{"coords": shards[i], "theta": theta} for i in range(N_CORES)]
    res = run_bass_kernel_spmd(nc, in_maps, list(range(N_CORES)))
    out = np.concatenate([res.results[i]["out"] for i in range(N_CORES)], axis=0)
    return out.reshape(N_TOTAL, 3)


# revision 16
# speedup vs baseline: 1.5325x; 1.5325x over previous
"""Trilinear interpolation (DeformationGrid) Bass kernel for 8 trn2 NeuronCores.

Data-parallel: coords/output sharded along the point axis across 8 cores;
theta replicated. Per core:
  1. Build a 2x2x2-corner block table in HBM (fp16): row b=(i*127+j)*127+k
     holds theta[i..i+1, j..j+1, k..k+1, :] (24 halves = 48B):
       - A16 slab: SWDGE cast-load (fp32 HBM -> fp16 SBUF), partition j =
         y-row j of x-slices [i0..i0+inn].
       - S16 slab: SWDGE SBUF->SBUF partition-shift (S16[j] = A16[j+1]) so
         y+1 data sits in partition j (DVE lanes can't cross partitions).
       - 4 strided DVE copies interleave (x,y)-corner segments into OT rows;
         a SWDGE DMA writes the rows to the HBM table.
     All bulk DMAs go through SWDGE (gpsimd): on this runtime the HWDGE
     path funnels every data descriptor through a single SDMA engine
     (~30 GB/s) while the other 15 engines' sem increments fire with no
     data behind them, releasing waiters early. SWDGE spreads descriptors
     across all 16 engines and each engine's increment follows its data.
  2. Stream points: cell ids + trilinear corner weights (DVE; fr=mod(xf,1)
     floor trick), ACT expands the 8 corner weights to 24 (one per gathered
     element), per-point 48B indirect-DMA gathers (one idx per partition
     per call - the HW consumes only the first index per partition), DVE
     fully-packed fp16 multiply + pairwise add tree.
Per-slot-parity DMA semaphores + issue gating make completion waits
unambiguous (a wait value can only be crossed by the DMA it guards).
Raw-bass implementation (manual semaphores).
"""
import sys

sys.path.insert(0, "/opt/trn_rl_repo")

import numpy as np

from concourse import bass, mybir
from concourse.bass import AP
from concourse.bass_utils import run_bass_kernel_spmd

N_CORES = 8
N_TOTAL = 4194304
NP = N_TOTAL // N_CORES        # 524288 points per core
G = 128
GD = G - 1                     # 127
NBLK = GD * GD * GD            # 2048383 block rows
ENT = 24                       # fp16 elems per block row (48B)
P = 128
B = 256                        # points per partition per stream tile
IW = 4                         # x-slices per build tile
WARM = 64                      # SWDGE warm-up gathers (cold-ring guard)
KL = 4                         # sub-DMAs per slab load
KW = 8                         # sub-DMAs per table write

F16 = mybir.dt.float16
F32 = mybir.dt.float32
I32 = mybir.dt.int32
OP = mybir.AluOpType
ACTF = mybir.ActivationFunctionType


def mkap(t_ap: AP, offset_elems: int, dims) -> AP:
    return AP(tensor=t_ap.tensor, offset=t_ap.offset + offset_elems, ap=list(dims))


def build_program(np_points: int = NP) -> bass.Bass:
    nc = bass.Bass()
    coords_in = nc.declare_dram_parameter("coords", [np_points, 3], F32, isOutput=False)
    theta_in = nc.declare_dram_parameter("theta", [G * G * G, 3], F32, isOutput=False)
    out_ext = nc.declare_dram_parameter("out", [np_points, 3], F32, isOutput=True)
    tbl = nc.dram_tensor("tbl", [NBLK, ENT], F16)

    T = np_points // (P * B)
    assert T * P * B == np_points
    bt = [(i0, min(IW, GD - i0)) for i0 in range(0, GD, IW)]
    NTB = len(bt)
    LDW = (IW + 1) * G * 3     # elems per partition per loaded slab

    from contextlib import ExitStack
    with ExitStack() as _ctx:
        A16 = _ctx.enter_context(nc.sbuf_tensor([P, 2 * LDW], F16))
        S16 = _ctx.enter_context(nc.sbuf_tensor([P, 2 * LDW], F16))
        OT = _ctx.enter_context(nc.sbuf_tensor([P, 2 * IW * GD * ENT], F16))
        CO = _ctx.enter_context(nc.sbuf_tensor([P, 2 * B * 3], F32))
        XF = _ctx.enter_context(nc.sbuf_tensor([P, 2 * B * 3], F32))
        FR = _ctx.enter_context(nc.sbuf_tensor([P, 2 * B * 3], F32))
        I0 = _ctx.enter_context(nc.sbuf_tensor([P, 2 * B * 3], F32))
        T1 = _ctx.enter_context(nc.sbuf_tensor([P, 2 * B], F32))
        T2 = _ctx.enter_context(nc.sbuf_tensor([P, 2 * B], F32))
        IDX = _ctx.enter_context(nc.sbuf_tensor([P, 2 * B], I32))
        W2X = _ctx.enter_context(nc.sbuf_tensor([P, 2 * B * 2], F16))
        W2Y = _ctx.enter_context(nc.sbuf_tensor([P, 2 * B * 2], F16))
        W2Z = _ctx.enter_context(nc.sbuf_tensor([P, 2 * B * 2], F16))
        M4 = _ctx.enter_context(nc.sbuf_tensor([P, 2 * B * 4], F16))
        W8 = _ctx.enter_context(nc.sbuf_tensor([P, 2 * B * 8], F16))
        W24 = _ctx.enter_context(nc.sbuf_tensor([P, 2 * B * ENT], F16))
        GT = _ctx.enter_context(nc.sbuf_tensor([P, 2 * B * ENT], F16))
        PR = _ctx.enter_context(nc.sbuf_tensor([P, B * ENT], F16))
        Q2 = _ctx.enter_context(nc.sbuf_tensor([P, B * 12], F16))
        R2 = _ctx.enter_context(nc.sbuf_tensor([P, B * 6], F16))
        O3 = _ctx.enter_context(nc.sbuf_tensor([P, 2 * B * 3], F32))
        TG = _ctx.enter_context(nc.sbuf_tensor([P, 2 * B * 3], F32))
        dmaL = [_ctx.enter_context(nc.semaphore(f"dmaL{i}")) for i in range(2)]
        dmaS = [_ctx.enter_context(nc.semaphore(f"dmaS{i}")) for i in range(2)]
        dmaW = _ctx.enter_context(nc.semaphore("dmaW"))
        dmaC = [_ctx.enter_context(nc.semaphore(f"dmaC{i}")) for i in range(2)]
        dmaO = [_ctx.enter_context(nc.semaphore(f"dmaO{i}")) for i in range(2)]
        dmaG = [_ctx.enter_context(nc.semaphore(f"dmaG{i}")) for i in range(8)]
        vec_b = _ctx.enter_context(nc.semaphore("vec_b"))
        v_fr = _ctx.enter_context(nc.semaphore("v_fr"))
        v_idx = _ctx.enter_context(nc.semaphore("v_idx"))
        v_w8 = _ctx.enter_context(nc.semaphore("v_w8"))
        act_s = _ctx.enter_context(nc.semaphore("act_s"))
        v_xf = _ctx.enter_context(nc.semaphore("v_xf"))
        v_o3 = _ctx.enter_context(nc.semaphore("v_o3"))
        dmaWm = _ctx.enter_context(nc.semaphore("dmaWm"))
        block = _ctx.enter_context(nc.Block())

        @block.sync
        def _(sync):
            # stream coords loads + output writes (HWDGE is slow on this
            # runtime but these are small and fully hidden under the
            # per-tile gather window)
            for t in range(T):
                if t >= 2:
                    sync.wait_ge(v_xf, t - 1)
                sync.dma_start(
                    out=mkap(CO[:], (t % 2) * B * 3, [[2 * B * 3, P], [1, B * 3]]),
                    in_=mkap(coords_in[:], t * P * B * 3, [[B * 3, P], [1, B * 3]]),
                ).then_inc(dmaC[t % 2], 16)
                if t >= 1:
                    sync.wait_ge(v_o3, t)
                    sync.dma_start(
                        out=mkap(out_ext[:], (t - 1) * P * B * 3,
                                 [[B * 3, P], [1, B * 3]]),
                        in_=mkap(O3[:], ((t - 1) % 2) * B * 3,
                                 [[2 * B * 3, P], [1, B * 3]]),
                    ).then_inc(dmaO[(t - 1) % 2], 16)
            sync.wait_ge(v_o3, T)
            sync.dma_start(
                out=mkap(out_ext[:], (T - 1) * P * B * 3, [[B * 3, P], [1, B * 3]]),
                in_=mkap(O3[:], ((T - 1) % 2) * B * 3, [[2 * B * 3, P], [1, B * 3]]),
            ).then_inc(dmaO[(T - 1) % 2], 16)
            sync.wait_ge(dmaO[(T - 1) % 2], 16 * ((T + 1) // 2))

        @block.scalar
        def _(scalar):
            # stream: W2[:, 0::2] = 1 - frac, W2[:, 1::2] = frac, then the
            # 8 corner weights expanded x3 into per-element weights W24
            for t in range(T):
                s3 = (t % 2) * B * 3
                s2 = (t % 2) * B * 2
                s8 = (t % 2) * B * 8
                sE = (t % 2) * B * ENT
                scalar.wait_ge(v_fr, t + 1)
                if t >= 2:
                    scalar.wait_ge(v_w8, t - 1)
                for c, W2 in ((0, W2X), (1, W2Y), (2, W2Z)):
                    nc.scalar.activation(
                        mkap(W2[:], s2, [[2 * B * 2, P], [2, B]]),
                        mkap(FR[:], s3 + c, [[2 * B * 3, P], [3, B]]),
                        ACTF.Copy, bias=1.0, scale=-1.0,
                    ).then_inc(act_s, 1)
                    nc.scalar.activation(
                        mkap(W2[:], s2 + 1, [[2 * B * 2, P], [2, B]]),
                        mkap(FR[:], s3 + c, [[2 * B * 3, P], [3, B]]),
                        ACTF.Copy,
                    ).then_inc(act_s, 1)
                scalar.wait_ge(v_w8, t + 1)
                if t >= 2:
                    scalar.wait_ge(v_o3, t - 1)
                nc.scalar.activation(
                    mkap(W24[:], sE, [[2 * B * ENT, P], [1, B * ENT]]),
                    mkap(W8[:], s8, [[2 * B * 8, P], [1, B * 8], [0, 3]]),
                    ACTF.Copy,
                ).then_inc(act_s, 1)

        @block.vector
        def _(vector):
            for bi, (i0, inn) in enumerate(bt):
                s = (bi % 2) * LDW
                so = (bi % 2) * IW * GD * ENT
                vector.wait_ge(dmaL[bi % 2], 16 * 2 * KL * (bi // 2 + 1))
                if bi >= 2:
                    vector.wait_ge(dmaW, 16 * KW * (bi - 1))
                for di, dj in ((0, 0), (0, 1), (1, 0), (1, 1)):
                    srcbuf = (A16 if dj == 0 else S16)
                    nc.vector.tensor_copy(
                        mkap(OT[:], so + (di * 2 + dj) * 6,
                             [[2 * IW * GD * ENT, GD], [GD * ENT, inn], [ENT, GD], [1, 6]]),
                        mkap(srcbuf[:], s + di * (G * 3),
                             [[2 * LDW, GD], [G * 3, inn], [3, GD], [1, 6]]),
                    ).then_inc(vec_b, 1)
            for t in range(T):
                s3 = (t % 2) * B * 3
                s1 = (t % 2) * B
                s2 = (t % 2) * B * 2
                s4 = (t % 2) * B * 4
                s8 = (t % 2) * B * 8
                vector.wait_ge(dmaC[t % 2], 16 * (t // 2 + 1))
                if t >= 2:
                    # IDX/GT slot reuse: gathers of t-2 must be fully issued
                    vector.wait_ge(dmaG[(t - 2) % 8], 16 * B * ((t - 2) // 8 + 1))
                if t >= 3:
                    # O3 slot reuse: the out-write that last read this slot
                    # (issued at sync iteration t-2) must be done
                    vector.wait_ge(dmaO[(t - 1) % 2],
                                   16 * ((t - 3 - ((t - 1) % 2)) // 2 + 1))
                co = mkap(CO[:], s3, [[2 * B * 3, P], [1, B * 3]])
                xf = mkap(XF[:], s3, [[2 * B * 3, P], [1, B * 3]])
                fr = mkap(FR[:], s3, [[2 * B * 3, P], [1, B * 3]])
                i0v = mkap(I0[:], s3, [[2 * B * 3, P], [1, B * 3]])
                tg = mkap(TG[:], s3, [[2 * B * 3, P], [1, B * 3]])
                nc.vector.tensor_scalar(xf, co, float(GD), None, OP.mult).then_inc(v_xf, 1)
                # floor(xf): round via magic number, then subtract (rounded > xf)
                nc.vector.tensor_scalar(i0v, xf, 8388608.0, 8388608.0, OP.add, OP.subtract)
                nc.vector.tensor_tensor(tg, i0v, xf, OP.is_gt)
                nc.vector.tensor_tensor(i0v, i0v, tg, OP.subtract)
                nc.vector.tensor_tensor(fr, xf, i0v, OP.subtract).then_inc(v_fr, 1)
                i03 = [mkap(I0[:], s3 + c, [[2 * B * 3, P], [3, B]]) for c in range(3)]
                t1 = mkap(T1[:], s1, [[2 * B, P], [1, B]])
                t2 = mkap(T2[:], s1, [[2 * B, P], [1, B]])
                idx = mkap(IDX[:], s1, [[2 * B, P], [1, B]])
                nc.vector.scalar_tensor_tensor(t1, i03[0], float(GD), i03[1], OP.mult, OP.add)
                nc.vector.scalar_tensor_tensor(t2, t1, float(GD), i03[2], OP.mult, OP.add)
                nc.vector.tensor_scalar(t2, t2, 0.0, float(NBLK - 1), OP.max, OP.min)
                nc.vector.tensor_copy(idx, t2).then_inc(v_idx, 1)
                # corner weight products: M4 = wx (x) wy, W8 = M4 (x) wz
                vector.wait_ge(act_s, 7 * t + 6)
                nc.vector.tensor_tensor(
                    mkap(M4[:], s4, [[2 * B * 4, P], [1, B * 4]]),
                    mkap(W2X[:], s2, [[2 * B * 2, P], [2, B], [1, 2], [0, 2]]),
                    mkap(W2Y[:], s2, [[2 * B * 2, P], [2, B], [0, 2], [1, 2]]),
                    OP.mult)
                nc.vector.tensor_tensor(
                    mkap(W8[:], s8, [[2 * B * 8, P], [1, B * 8]]),
                    mkap(M4[:], s4, [[2 * B * 4, P], [4, B], [1, 4], [0, 2]]),
                    mkap(W2Z[:], s2, [[2 * B * 2, P], [2, B], [0, 4], [1, 2]]),
                    OP.mult).then_inc(v_w8, 1)
                if t >= 1:
                    tp = t - 1
                    sEp = (tp % 2) * B * ENT
                    s3p = (tp % 2) * B * 3
                    vector.wait_ge(dmaG[tp % 8], 16 * B * (tp // 8 + 1))
                    vector.wait_ge(act_s, 7 * (tp + 1))
                    nc.vector.tensor_tensor(
                        mkap(PR[:], 0, [[B * ENT, P], [1, B * ENT]]),
                        mkap(GT[:], sEp, [[2 * B * ENT, P], [1, B * ENT]]),
                        mkap(W24[:], sEp, [[2 * B * ENT, P], [1, B * ENT]]),
                        OP.mult)
                    # pairwise add tree over the 8 corners (xy pairs, then z)
                    nc.vector.tensor_tensor(
                        mkap(Q2[:], 0, [[B * 12, P], [1, B * 12]]),
                        mkap(PR[:], 0, [[B * ENT, P], [ENT, B], [1, 12]]),
                        mkap(PR[:], 12, [[B * ENT, P], [ENT, B], [1, 12]]),
                        OP.add)
                    nc.vector.tensor_tensor(
                        mkap(R2[:], 0, [[B * 6, P], [1, B * 6]]),
                        mkap(Q2[:], 0, [[B * 12, P], [12, B], [1, 6]]),
                        mkap(Q2[:], 6, [[B * 12, P], [12, B], [1, 6]]),
                        OP.add)
                    nc.vector.tensor_tensor(
                        mkap(O3[:], s3p, [[2 * B * 3, P], [1, B * 3]]),
                        mkap(R2[:], 0, [[B * 6, P], [6, B], [1, 3]]),
                        mkap(R2[:], 3, [[B * 6, P], [6, B], [1, 3]]),
                        OP.add).then_inc(v_o3, 1)
            tp = T - 1
            sEp = (tp % 2) * B * ENT
            s3p = (tp % 2) * B * 3
            vector.wait_ge(dmaG[tp % 8], 16 * B * (tp // 8 + 1))
            vector.wait_ge(act_s, 7 * (tp + 1))
            nc.vector.tensor_tensor(
                mkap(PR[:], 0, [[B * ENT, P], [1, B * ENT]]),
                mkap(GT[:], sEp, [[2 * B * ENT, P], [1, B * ENT]]),
                mkap(W24[:], sEp, [[2 * B * ENT, P], [1, B * ENT]]),
                OP.mult)
            nc.vector.tensor_tensor(
                mkap(Q2[:], 0, [[B * 12, P], [1, B * 12]]),
                mkap(PR[:], 0, [[B * ENT, P], [ENT, B], [1, 12]]),
                mkap(PR[:], 12, [[B * ENT, P], [ENT, B], [1, 12]]),
                OP.add)
            nc.vector.tensor_tensor(
                mkap(R2[:], 0, [[B * 6, P], [1, B * 6]]),
                mkap(Q2[:], 0, [[B * 12, P], [12, B], [1, 6]]),
                mkap(Q2[:], 6, [[B * 12, P], [12, B], [1, 6]]),
                OP.add)
            nc.vector.tensor_tensor(
                mkap(O3[:], s3p, [[2 * B * 3, P], [1, B * 3]]),
                mkap(R2[:], 0, [[B * 6, P], [6, B], [1, 3]]),
                mkap(R2[:], 3, [[B * 6, P], [6, B], [1, 3]]),
                OP.add).then_inc(v_o3, 1)

        def table_write(gpsimd, bi):
            # split across KW sub-DMAs: consecutive SWDGE instructions land
            # on different SDMA engines (a single DMA's data descriptors all
            # go to ONE engine on this runtime)
            p0, pn = bt[bi]
            so = (bi % 2) * IW * GD * ENT
            for w in range(KW):
                j0 = (GD * w) // KW
                j1 = (GD * (w + 1)) // KW
                gpsimd.dma_start(
                    out=mkap(tbl[:], p0 * GD * GD * ENT + j0 * GD * ENT,
                             [[GD * ENT, j1 - j0], [GD * GD * ENT, pn], [1, GD * ENT]]),
                    in_=mkap(OT[:], so + j0 * (2 * IW * GD * ENT),
                             [[2 * IW * GD * ENT, j1 - j0], [GD * ENT, pn], [1, GD * ENT]]),
                ).then_inc(dmaW, 16)

        @block.gpsimd
        def _(gpsimd):
            # build: split SWDGE cast-loads (A16: y-rows j, S16: y-rows j+1
            # loaded from HBM with a one-row offset), split table writes
            for bi, (i0, inn) in enumerate(bt):
                nld = inn + 1
                s = (bi % 2) * LDW
                if bi >= 2:
                    gpsimd.wait_ge(vec_b, 4 * (bi - 1))
                for l in range(KL):
                    q0 = (P * l) // KL
                    q1 = (P * (l + 1)) // KL
                    gpsimd.dma_start(
                        out=mkap(A16[:], s + q0 * (2 * LDW),
                                 [[2 * LDW, q1 - q0], [G * 3, nld], [1, G * 3]]),
                        in_=mkap(theta_in[:], i0 * G * G * 3 + q0 * G * 3,
                                 [[G * 3, q1 - q0], [G * G * 3, nld], [1, G * 3]]),
                    ).then_inc(dmaL[bi % 2], 16)
                for l in range(KL):
                    q0 = (GD * l) // KL
                    q1 = (GD * (l + 1)) // KL
                    gpsimd.dma_start(
                        out=mkap(S16[:], s + q0 * (2 * LDW),
                                 [[2 * LDW, q1 - q0], [G * 3, nld], [1, G * 3]]),
                        in_=mkap(theta_in[:], (i0 * G + 1) * G * 3 + q0 * G * 3,
                                 [[G * 3, q1 - q0], [G * G * 3, nld], [1, G * 3]]),
                    ).then_inc(dmaL[bi % 2], 16)
                if bi >= 1:
                    gpsimd.wait_ge(vec_b, 4 * bi)
                    table_write(gpsimd, bi - 1)
            gpsimd.wait_ge(vec_b, 4 * NTB)
            table_write(gpsimd, NTB - 1)
            gpsimd.wait_ge(dmaW, 16 * KW * NTB)
            # SWDGE warm-up: duplicate tile-0 gathers (cold rings corrupt
            # the first batch on engine 0 otherwise); real tile 0 follows
            gpsimd.wait_ge(v_idx, 1)
            for k in range(WARM):
                gpsimd.indirect_dma_start(
                    out=mkap(GT[:], k * ENT, [[2 * B * ENT, P], [1, ENT]]),
                    out_offset=None,
                    in_=tbl[:],
                    in_offset=bass.IndirectOffsetOnAxis(
                        ap=mkap(IDX[:], k, [[2 * B, P], [1, 1]]), axis=0),
                ).then_inc(dmaWm, 16)
            gpsimd.wait_ge(dmaWm, 16 * WARM)
            for t in range(T):
                sE = (t % 2) * B * ENT
                s1 = (t % 2) * B
                gpsimd.wait_ge(v_idx, t + 1)
                if t >= 2:
                    gpsimd.wait_ge(v_o3, t - 1)
                for k in range(B):
                    gpsimd.indirect_dma_start(
                        out=mkap(GT[:], sE + k * ENT, [[2 * B * ENT, P], [1, ENT]]),
                        out_offset=None,
                        in_=tbl[:],
                        in_offset=bass.IndirectOffsetOnAxis(
                            ap=mkap(IDX[:], s1 + k, [[2 * B, P], [1, 1]]), axis=0),
                    ).then_inc(dmaG[t % 8], 16)
    return nc


_CACHED = {}


def _get_program():
    if "nc" not in _CACHED:
        _CACHED["nc"] = build_program()
    return _CACHED["nc"]


def kernel(coords: np.ndarray, theta: np.ndarray) -> np.ndarray:
    coords = np.ascontiguousarray(coords, dtype=np.float32)
    theta = np.ascontiguousarray(theta, dtype=np.float32).reshape(G * G * G, 3)
    nc = _get_program()
    shards = coords.reshape(N_CORES, NP, 3)
    in_maps = [{"coords": shards[i], "theta": theta} for i in range(N_CORES)]
    res = run_bass_kernel_spmd(nc, in_maps, list(range(N_CORES)))
    out = np.concatenate([res.results[i]["out"] for i in range(N_CORES)], axis=0)
    return out.reshape(N_TOTAL, 3)
